# revision 30
# baseline (speedup 1.0000x reference)
"""Trainium2 Bass kernel for an encoder-decoder (S2S) transformer.

Distribution: 8 NeuronCores = 4 data-parallel groups (batch B=4) x 2-way
Megatron tensor-parallel pairs.  Per TP pair, attention heads (qkv/out) and
ffn (ff1/ff2) are sharded; partials combine with a 2-core AllReduce after
the attention out-projection and after ff2.  The output head is split by
vocab half across the pair.

Matmuls run in bf16 on the TensorEngine (fp32 PSUM accumulation); the
residual stream is kept in bf16 (LN statistics in fp32).  Activations are
SBUF-resident for the whole pass; only weights stream from HBM.

Pipelining: every sublayer boundary is processed in two token halves.  The
AllReduce for half h is issued as soon as its producer matmuls finish, and
the LN + transpose + next-sublayer matmuls of half 0 execute while half 1's
AllReduce is still in flight, keeping the TensorEngine fed (the PE clock
ramps to full speed only after ~3us of continuous execution, so gaps are
doubly expensive).

Layout conventions (per core, P=128, T=512, HT=256):
  x_bf[t]    [P, D]      bf16  residual stream, tokens-on-partitions
  x_T_h[hf]  [P, ND*HT]  bf16  transposed half: chunk d cols [d*HT:(d+1)*HT]
  q_T/k_T    [P, NQK*T]  bf16  rows = sharded head dims, chunk m = heads 2m,2m+1
  v_aug[t]   [P, HL*128] bf16  per head 64 value cols + 64 ones cols
  attn_T     [P, NO*T]   bf16  context, transposed (rows = sharded dims)
  h_T_h[hf]  [P, NFF*HT] bf16  ffn hidden, transposed half
"""

import os
import sys

for _p in ("/opt/trn_rl_repo", "/root/.axon_site/_ro/trn_rl_repo"):
    if os.path.isdir(_p) and _p not in sys.path:
        sys.path.insert(0, _p)

import numpy as np
import ml_dtypes

import concourse.bass as bass
import concourse.bacc as bacc
import concourse.tile as tile
from concourse import mybir
from concourse.bass import IndirectOffsetOnAxis
from concourse.masks import make_identity

BF16 = ml_dtypes.bfloat16
F32 = mybir.dt.float32
BF = mybir.dt.bfloat16
I32 = mybir.dt.int32
AF = mybir.ActivationFunctionType
ALU = mybir.AluOpType
AX = mybir.AxisListType

P = 128


class Cfg:
    def __init__(self, B, Q, T, D, H, V, L, FF, TP, n_cores, flags=frozenset()):
        self.B, self.Q, self.T, self.D, self.H, self.V, self.L, self.FF = \
            B, Q, T, D, H, V, L, FF
        self.TP, self.n_cores = TP, n_cores
        self.E = D // Q
        self.HD = D // H
        assert self.HD == 64, "head packing assumes head_dim 64"
        assert self.E == P, "per-quantizer embedding dim must be 128"
        self.SCALE = 1.0 / float(np.sqrt(self.HD))
        self.DL = D // TP
        self.FFL = FF // TP
        self.HL = H // TP
        self.V2 = V // TP
        self.NT = T // P
        self.HT = self.T // 2
        self.ND = D // P
        self.NQK = self.DL // P
        self.NO = self.DL // P
        self.NFF = self.FFL // P
        assert self.HL % 2 == 0, "needs an even number of local heads"
        assert self.NT == 4, "token-half pipelining assumes 4 token blocks"
        self.flags = frozenset(flags)

    def key(self):
        return (self.B, self.Q, self.T, self.D, self.H, self.V, self.L,
                self.FF, self.TP, self.n_cores, tuple(sorted(self.flags)))


# --------------------------------------------------------------------------
# program builder
# --------------------------------------------------------------------------

def build_program(c: Cfg):
    nc = bacc.Bacc(None, target_bir_lowering=False, num_devices=c.n_cores)

    def din(name, shape, dt=BF):
        return nc.dram_tensor(name, shape, dt, kind="ExternalInput")

    codes_in = din("codes_in", [c.Q, c.T], I32)
    codes_tgt = din("codes_tgt", [c.Q, c.T], I32)
    tok_emb = [din(f"tok_emb_{q}", [c.V, c.E], F32) for q in range(c.Q)]
    pos = din("pos", [c.T, c.D], F32)

    w = {}
    for l in range(c.L):
        for nm, sh in [
                (f"e_qkv_{l}", [c.D, 3 * c.DL]), (f"e_out_{l}", [c.DL, c.D]),
                (f"e_ff2_{l}", [c.FFL, c.D]),
                (f"d_sqkv_{l}", [c.D, 3 * c.DL]), (f"d_sout_{l}", [c.DL, c.D]),
                (f"d_cqkv_{l}", [c.D, 3 * c.DL]), (f"d_cout_{l}", [c.DL, c.D]),
                (f"d_ff2_{l}", [c.FFL, c.D])]:
            w[nm] = din(nm, sh)
        # pre-tiled stationary weights: one contiguous [P, ND*P] block per
        # matmul tile so each weight DMA is a single cheap descriptor
        for nm in ("e_qkv", "d_sqkv", "d_cqkv"):
            w[f"{nm}_qk_{l}"] = din(f"{nm}_qk_{l}",
                                    [2 * c.NQK, P, c.ND * P])
        for nm in ("e_ff1", "d_ff1"):
            w[f"{nm}_t_{l}"] = din(f"{nm}_t_{l}", [c.NFF, P, c.ND * P])
    w["head_t"] = din("head_t", [c.E, c.Q * c.V2])

    opt = {}
    for nm in c.flags:
        if "_qkv_b_" in nm or "_sqkv_b_" in nm or "_cqkv_b_" in nm:
            opt[nm] = din(nm, [3 * c.DL], F32)
        elif "_ff1_b_" in nm:
            opt[nm] = din(nm, [c.FFL], F32)
        elif nm == "head_b":
            opt[nm] = din(nm, [P, c.Q * c.V2], F32)
        else:
            opt[nm] = din(nm, [P, c.D], F32)

    logits = nc.dram_tensor("logits", [c.Q, c.T, c.V2], F32,
                            kind="ExternalOutput")

    groups = ([[g * c.TP + i for i in range(c.TP)]
               for g in range(c.n_cores // c.TP)] if c.TP > 1 else None)

    with tile.TileContext(nc) as tc:
        _emit(nc, tc, c, codes_in, codes_tgt, tok_emb, pos, w, opt, logits,
              groups)
    nc.compile()
    return nc


def _emit(nc, tc, c, codes_in, codes_tgt, tok_emb, pos, w, opt, logits,
          groups):
    from contextlib import ExitStack
    es = ExitStack()
    pool = lambda name, bufs, space="SBUF": es.enter_context(
        tc.tile_pool(name=name, bufs=bufs, space=space))

    const = pool("const", 1)
    persist = pool("persist", 1)
    wpool = pool("wpool", 3)
    scratch = pool("scratch", 2)
    atp = pool("atp", 2)
    dram = pool("dram", 4, space="DRAM")
    ps_proj = pool("ps_proj", 2, space="PSUM")
    ps_s = pool("ps_s", 2, space="PSUM")
    ps_av = pool("ps_av", 2, space="PSUM")
    ps_tr = pool("ps_tr", 2, space="PSUM")

    HT = c.HT

    # constants
    ident_b = const.tile([P, P], BF, name="ident_b")
    make_identity(nc, ident_b[:])
    causT = const.tile([P, P], F32, name="causT")
    nc.gpsimd.memset(causT[:], 0.0)
    # transposed causal: fill -1e9 where tk > tq  (iota = tq - tk >= 0 keeps)
    nc.gpsimd.affine_select(out=causT[:], in_=causT[:],
                            compare_op=ALU.is_ge, fill=-1e9, base=0,
                            pattern=[[1, P]], channel_multiplier=-1)

    eps_t = const.tile([P, 1], F32, name="eps_t")
    nc.vector.memset(eps_t[:], 1e-5)

    opt_sb = {}
    for nm in opt:
        if "qkv_b_" in nm:
            t = const.tile([P, 3 * c.NQK], F32, name=f"sb_{nm}")
            nc.sync.dma_start(out=t[:],
                              in_=opt[nm].rearrange("(m p) -> p m", p=P))
        elif "_ff1_b_" in nm:
            t = const.tile([P, c.NFF], F32, name=f"sb_{nm}")
            nc.sync.dma_start(out=t[:],
                              in_=opt[nm].rearrange("(m p) -> p m", p=P))
        else:
            sh = [P, c.Q * c.V2] if nm == "head_b" else [P, c.D]
            t = const.tile(sh, F32, name=f"sb_{nm}")
            nc.sync.dma_start(out=t[:], in_=opt[nm][:])
        opt_sb[nm] = t

    # persistent activations
    x_bf = [persist.tile([P, c.D], BF, name=f"xb_{t}", tag=f"xb_{t}")
            for t in range(c.NT)]
    x_T = persist.tile([P, c.ND * c.T], BF, name="x_T", tag="x_T")
    mem_T = persist.tile([P, c.ND * c.T], BF, name="mem_T", tag="mem_T")
    q_T = persist.tile([P, c.NQK * c.T], BF, name="q_T", tag="q_T")
    k_T = persist.tile([P, c.NQK * c.T], BF, name="k_T", tag="k_T")
    WA = c.HL * 128   # per-t block: per head 64 ones cols + 64 value cols
    v_aug = [persist.tile([P, WA], BF, name=f"va_{t}", tag=f"va_{t}")
             for t in range(c.NT)]
    attn_T = persist.tile([P, c.NO * c.T], BF, name="attn_T", tag="attn_T")
    h_T = persist.tile([P, c.NFF * c.T], BF, name="h_T", tag="h_T")

    # ones columns of v_aug are set once; projections only write data cols.
    # ones come FIRST so the softmax denominator lands on partitions 0..63
    # (reciprocal_approx_fast needs a partition-aligned input).
    for t in range(c.NT):
        blk = v_aug[t][:].rearrange("p (h f) -> p h f", f=128)
        nc.vector.memset(blk[:, :, 0:64], 1.0)

    # psum-drain copies: route to the engine that is idle in that phase
    _cp = [0]

    def copy_out(dst, src, eng=None):
        if eng is None:
            eng = "s" if _cp[0] % 2 == 0 else "v"
            _cp[0] += 1
        if eng == "s":
            nc.scalar.copy(out=dst, in_=src)
        elif eng == "g":
            nc.gpsimd.tensor_copy(out=dst, in_=src)
        else:
            nc.vector.tensor_copy(out=dst, in_=src)

    # ---------------- helpers ----------------
    def transpose_tokens(dst, hf):
        """dst[:, d*T + t*P : +P] = x_bf[t][:, d*P:+P].T"""
        dv = dst[:].rearrange("p (nd x) -> p nd x", x=c.T)
        for t in (2 * hf, 2 * hf + 1):
            for dh in range(c.ND // 4):
                pt = ps_tr.tile([P, 4 * P], BF, tag="ps_tr", name="pt")
                for j in range(4):
                    d = 4 * dh + j
                    nc.tensor.transpose(pt[:, j * P:(j + 1) * P],
                                        x_bf[t][:, d * P:(d + 1) * P],
                                        ident_b[:])
                copy_out(dv[:, 4 * dh:4 * dh + 4, t * P:(t + 1) * P],
                         pt[:].rearrange("p (d x) -> p d x", x=P))

    def embed(codes):
        for t in range(c.NT):
            ptile = scratch.tile([P, c.D], F32, tag="pos", name="ptile",
                                 bufs=1)
            nc.sync.dma_start(out=ptile[:], in_=pos[t * P:(t + 1) * P, :])
            e_t = scratch.tile([P, c.D], F32, tag="emb", name="e_t", bufs=1)
            for q in range(c.Q):
                idx = scratch.tile([P, 1], I32, tag="idx", name="idx", bufs=8)
                nc.sync.dma_start(out=idx[:], in_=codes[q, t * P:(t + 1) * P])
                nc.gpsimd.indirect_dma_start(
                    out=e_t[:, q * c.E:(q + 1) * c.E],
                    out_offset=None,
                    in_=tok_emb[q][:],
                    in_offset=IndirectOffsetOnAxis(ap=idx[:, :1], axis=0))
            nc.vector.tensor_tensor(out=e_t[:], in0=e_t[:], in1=ptile[:],
                                    op=ALU.add)
            copy_out(x_bf[t][:], e_t[:], eng="s")

    def ln_chunk(t, sub, gname, bname):
        """x_bf[t] = LN(x_bf[t] + sub) * g + b (post-norm), bf16 residual."""
        tmp = scratch.tile([P, c.D], F32, tag="lntmp", name="tmp", bufs=2)
        nc.vector.tensor_tensor(out=tmp[:], in0=x_bf[t][:], in1=sub,
                                op=ALU.add)
        s1 = scratch.tile([P, 1], F32, tag="lnstat", name="s1", bufs=8)
        s2 = scratch.tile([P, 1], F32, tag="lnstat", name="s2", bufs=8)
        sq = scratch.tile([P, c.D], BF, tag="lnsq", name="sq", bufs=2)
        nc.vector.reduce_sum(out=s1[:], in_=tmp[:], axis=AX.X)
        nc.scalar.activation(sq[:], tmp[:], AF.Square, accum_out=s2[:])
        mean = scratch.tile([P, 1], F32, tag="lnstat", name="mean", bufs=8)
        var = scratch.tile([P, 1], F32, tag="lnstat", name="var", bufs=8)
        m2 = scratch.tile([P, 1], F32, tag="lnstat", name="m2", bufs=8)
        nc.vector.tensor_scalar_mul(mean[:], s1[:], 1.0 / c.D)
        nc.vector.tensor_scalar_mul(var[:], s2[:], 1.0 / c.D)
        nc.vector.tensor_tensor(out=m2[:], in0=mean[:], in1=mean[:],
                                op=ALU.mult)
        nc.vector.tensor_tensor(out=var[:], in0=var[:], in1=m2[:],
                                op=ALU.subtract)
        rstd = scratch.tile([P, 1], F32, tag="lnstat", name="rstd", bufs=8)
        nc.scalar.activation(rstd[:], var[:], AF.Sqrt, bias=eps_t[:])
        nc.vector.reciprocal(rstd[:], rstd[:])
        if gname in opt_sb or bname in opt_sb:
            nc.vector.tensor_scalar(out=tmp[:], in0=tmp[:], scalar1=mean[:],
                                    scalar2=rstd[:], op0=ALU.subtract,
                                    op1=ALU.mult)
            if gname in opt_sb:
                nc.vector.tensor_tensor(out=tmp[:], in0=tmp[:],
                                        in1=opt_sb[gname][:], op=ALU.mult)
            if bname in opt_sb:
                nc.vector.tensor_tensor(out=tmp[:], in0=tmp[:],
                                        in1=opt_sb[bname][:], op=ALU.add)
            nc.vector.tensor_copy(out=x_bf[t][:], in_=tmp[:])
        else:
            nc.vector.tensor_scalar(out=x_bf[t][:], in0=tmp[:],
                                    scalar1=mean[:], scalar2=rstd[:],
                                    op0=ALU.subtract, op1=ALU.mult)

    red_tiles = [None, None]

    def ar_start(ph, hf):
        """Kick off the TP AllReduce for token half hf of a sublayer."""
        if groups is None:
            red_tiles[hf] = ph
            return
        arin = dram.tile([P, 2 * c.D], BF, tag="arin", name="arin", bufs=4)
        arout = dram.tile([P, 2 * c.D], BF, tag="arout", name="arout", bufs=4)
        nc.sync.dma_start(out=arin[:], in_=ph[:])
        nc.gpsimd.collective_compute(
            "AllReduce", ALU.add, replica_groups=groups,
            ins=[arin[:].opt()], outs=[arout[:].opt()])
        red = scratch.tile([P, 2 * c.D], BF, tag="ar_red", name="red", bufs=2)
        nc.sync.dma_start(out=red[:], in_=arout[:])
        red_tiles[hf] = red

    def ln_finish(hf, gname, bname):
        red = red_tiles[hf]
        for i, t in enumerate((2 * hf, 2 * hf + 1)):
            ln_chunk(t, red[:, i * c.D:(i + 1) * c.D], gname, bname)

    def boundary(gname, bname, dst_pair, next_fn):
        """AR results -> LN -> transpose -> next sublayer, per token half."""
        for hf in range(2):
            ln_finish(hf, gname, bname)
            transpose_tokens(dst_pair, hf)
            if next_fn is not None:
                next_fn(hf)

    def load_qk(tname, n0=0, n1=None):
        """Load pre-tiled stationary weight blocks (one DMA per tile)."""
        if n1 is None:
            n1 = 2 * c.NQK
        tiles = {}
        for j in range(n0, n1):
            wt = wpool.tile([P, c.ND * P], BF, tag=f"wqk_{j}", name="wt",
                            bufs=1)
            nc.sync.dma_start(out=wt[:], in_=w[tname][j])
            tiles[j] = wt
        return tiles

    def proj_half(qk_tiles, bname, src, row_base, dst, hf):
        """dst[:, m*T + hf*HT : +HT] = rows [row_base + m*P ..] of W.T @ src."""
        for m in range(c.NQK):
            mg = row_base // P + m
            wt = qk_tiles[mg]
            ps = ps_proj.tile([P, 512], F32, tag="ps_proj", name="ps")
            for k in range(c.ND):
                nc.tensor.matmul(ps[:, :HT], wt[:, k * P:(k + 1) * P],
                                 src[:, k * c.T + hf * HT:
                                     k * c.T + (hf + 1) * HT],
                                 start=(k == 0), stop=(k == c.ND - 1))
            col = m * c.T + hf * HT
            if bname in opt_sb:
                nc.vector.tensor_scalar(
                    out=dst[:, col:col + HT], in0=ps[:, :HT],
                    scalar1=opt_sb[bname][:, mg:mg + 1], scalar2=None,
                    op0=ALU.add)
            else:
                copy_out(dst[:, col:col + HT], ps[:, :HT], eng="v")

    def proj_full(qk_tiles, bname, src, row_base, dst):
        for m in range(c.NQK):
            mg = row_base // P + m
            wt = qk_tiles[mg]
            ps = ps_proj.tile([P, 512], F32, tag="ps_proj", name="ps")
            for k in range(c.ND):
                nc.tensor.matmul(ps[:], wt[:, k * P:(k + 1) * P],
                                 src[:, k * c.T:(k + 1) * c.T],
                                 start=(k == 0), stop=(k == c.ND - 1))
            col = m * c.T
            if bname in opt_sb:
                nc.vector.tensor_scalar(
                    out=dst[:, col:col + c.T], in0=ps[:],
                    scalar1=opt_sb[bname][:, mg:mg + 1], scalar2=None,
                    op0=ALU.add)
            else:
                copy_out(dst[:, col:col + c.T], ps[:], eng="v")

    def proj_v_half(src, hf, wv_tiles):
        for t in (2 * hf, 2 * hf + 1):
            ps = ps_proj.tile([P, 512], F32, tag="ps_proj", name="ps")
            for k in range(c.ND):
                nc.tensor.matmul(
                    ps[:, :c.DL],
                    src[:, k * c.T + t * P: k * c.T + t * P + P],
                    wv_tiles[k][:], start=(k == 0), stop=(k == c.ND - 1))
            blk = v_aug[t][:].rearrange("p (h f) -> p h f", f=128)
            copy_out(blk[:, :, 64:128],
                     ps[:, :c.DL].rearrange("p (h f) -> p h f", f=64))

    def load_wv(wname):
        wv_tiles = []
        for k in range(c.ND):
            wt = wpool.tile([P, c.DL], BF, tag=f"wv_{k}", name="wt", bufs=1)
            nc.sync.dma_start(
                out=wt[:],
                in_=w[wname][k * P:(k + 1) * P, 2 * c.DL:3 * c.DL])
            wv_tiles.append(wt)
        return wv_tiles

    def qkv_fn(wname, bname):
        state = {}
        base, l = wname.rsplit("_", 1)

        def load():
            if "qk" not in state:
                state["qk"] = load_qk(f"{base}_qk_{l}")
                state["wv"] = load_wv(wname)

        def go(hf):
            if hf == 0:
                load()
            proj_half(state["qk"], bname, x_T, 0, q_T, hf)
            proj_half(state["qk"], bname, x_T, c.DL, k_T, hf)
            proj_v_half(x_T, hf, state["wv"])
        go.load = load
        return go

    def attention(causal):
        """q_T,k_T,v_aug -> attn_T, pipelined one head deep so the PE does
        head h+1's scores while the scalar engine exponentiates head h."""
        def emit_scores(hh):
            m, po = hh // 2, 64 * (hh % 2)
            ats = []
            for tk in range(c.NT):
                tq0 = tk * P if causal else 0
                pss = ps_s.tile([P, c.T], F32, tag="ps_s", name="pss")
                at = atp.tile([P, c.T], BF, tag=f"AT{tk}", name="at", bufs=2)
                nc.tensor.matmul(
                    pss[:, tq0:c.T],
                    k_T[po:po + 64, m * c.T + tk * P: m * c.T + tk * P + P],
                    q_T[po:po + 64, m * c.T + tq0: m * c.T + c.T],
                    start=True, stop=True)
                if causal:
                    nc.vector.tensor_tensor(
                        out=pss[:, tk * P:(tk + 1) * P],
                        in0=pss[:, tk * P:(tk + 1) * P],
                        in1=causT[:], op=ALU.add)
                nc.scalar.activation(at[:, tq0:c.T], pss[:, tq0:c.T],
                                     AF.Exp, scale=c.SCALE)
                ats.append((at, tq0))
            return ats

        def emit_av(hh, ats):
            m, po = hh // 2, 64 * (hh % 2)
            ps_o = ps_av.tile([P, c.T], F32, tag="ps_av", name="ps_o")
            for tk in range(c.NT):
                cols0 = ats[tk][1]
                nc.tensor.matmul(
                    ps_o[:, cols0:c.T],
                    v_aug[tk][:, hh * 128:(hh + 1) * 128],
                    ats[tk][0][:, cols0:c.T],
                    start=(tk == 0), stop=(tk == c.NT - 1))
            rden = scratch.tile([64, c.T], F32, tag="rden", name="rden",
                                bufs=3)
            nc.vector.reciprocal_approx_fast(rden[:], ps_o[0:64, :])
            nc.vector.tensor_tensor(
                out=attn_T[po:po + 64, m * c.T:(m + 1) * c.T],
                in0=ps_o[64:128, :], in1=rden[:], op=ALU.mult)

        prev = None
        for hh in range(c.HL):
            ats = emit_scores(hh)
            if prev is not None:
                emit_av(*prev)
            prev = (hh, ats)
        emit_av(*prev)

    NB = max(c.D // 512, 1)
    NW = min(512, c.D)

    def natural_half_ar(src_T_of, nk, wts, bname, hf):
        """parts[tokens of half hf, :D] = src^T @ W; then start the AR."""
        ph = scratch.tile([P, 2 * c.D], BF, tag="oproj", name="ph", bufs=2)
        for i, t in enumerate((2 * hf, 2 * hf + 1)):
            for n in range(NB):
                ps = ps_proj.tile([P, 512], F32, tag="ps_proj", name="ps")
                for k in range(nk):
                    nc.tensor.matmul(
                        ps[:], src_T_of(k, t),
                        wts[k][:, n * NW:(n + 1) * NW],
                        start=(k == 0), stop=(k == nk - 1))
                copy_out(ph[:, i * c.D + n * NW: i * c.D + (n + 1) * NW],
                         ps[:], eng="v")
        if bname in opt_sb:
            for i in range(2):
                nc.vector.tensor_tensor(
                    out=ph[:, i * c.D:(i + 1) * c.D],
                    in0=ph[:, i * c.D:(i + 1) * c.D],
                    in1=opt_sb[bname][:], op=ALU.add)
        ar_start(ph, hf)

    def load_out(wname):
        wts = []
        for k in range(c.NO):
            wt = wpool.tile([P, c.D], BF, tag=f"wo_{k}", name="wt", bufs=1)
            nc.sync.dma_start(out=wt[:], in_=w[wname][k * P:(k + 1) * P, :])
            wts.append(wt)
        return wts

    def out_proj_ar(wts, bname):
        src = lambda k, t: attn_T[:, k * c.T + t * P: k * c.T + t * P + P]
        for hf in range(2):
            natural_half_ar(src, c.NO, wts, bname, hf)

    NW1 = 6   # ff1 weight-tile rotation width

    def ffn_fn(w1name, b1name, w2name, b2name):
        state = {}
        base, l = w1name.rsplit("_", 1)
        w1t = f"{base}_t_{l}"

        def load():
            if "pre" in state:
                return
            # prefetch the first ff1 tiles, then the resident ff2 tiles, so
            # both transfer during the attention-out AllReduce
            pre = []
            for m in range(NW1):
                wt = wpool.tile([P, c.ND * P], BF, tag=f"wf1_{m % NW1}",
                                name="wt", bufs=2)
                nc.sync.dma_start(out=wt[:], in_=w[w1t][m])
                pre.append(wt)
            state["pre"] = pre
            wts = []
            for k in range(c.NFF):
                wt = wpool.tile([P, c.D], BF, tag=f"wff2_{k}", name="wt",
                                bufs=1)
                nc.sync.dma_start(out=wt[:],
                                  in_=w[w2name][k * P:(k + 1) * P, :])
                wts.append(wt)
            state["w2"] = wts

        def go(hf):
            if hf == 0:
                load()
            for m in range(c.NFF):
                if hf == 0 and m < NW1:
                    wt = state["pre"][m]
                else:
                    wt = wpool.tile([P, c.ND * P], BF, tag=f"wf1_{m % NW1}",
                                    name="wt", bufs=2)
                    nc.sync.dma_start(out=wt[:], in_=w[w1t][m])
                ps = ps_proj.tile([P, 512], F32, tag="ps_proj", name="ps")
                for k in range(c.ND):
                    nc.tensor.matmul(ps[:, :HT], wt[:, k * P:(k + 1) * P],
                                     x_T[:, k * c.T + hf * HT:
                                         k * c.T + (hf + 1) * HT],
                                     start=(k == 0), stop=(k == c.ND - 1))
                col = m * c.T + hf * HT
                if b1name in opt_sb:
                    nc.scalar.activation(h_T[:, col:col + HT],
                                         ps[:, :HT], AF.Relu,
                                         bias=opt_sb[b1name][:, m:m + 1])
                else:
                    nc.scalar.activation(h_T[:, col:col + HT],
                                         ps[:, :HT], AF.Relu)
            src = lambda k, t: h_T[:, k * c.T + t * P: k * c.T + t * P + P]
            natural_half_ar(src, c.NFF, state["w2"], b2name, hf)
        go.load = load
        return go

    def head_fn(hf):
        for j in range(c.Q):
            hw = wpool.tile([P, c.V2], BF, tag="whead", name="hw", bufs=2)
            nc.sync.dma_start(out=hw[:],
                              in_=w["head_t"][:, j * c.V2:(j + 1) * c.V2])
            for t in (2 * hf, 2 * hf + 1):
                ps = ps_proj.tile([P, 512], F32, tag="ps_proj", name="ps")
                nc.tensor.matmul(
                    ps[:, :c.V2],
                    x_T[:, j * c.T + t * P: j * c.T + t * P + P],
                    hw[:], start=True, stop=True)
                sb = scratch.tile([P, c.V2], F32, tag="lgt", name="sb",
                                  bufs=2)
                copy_out(sb[:], ps[:, :c.V2])
                if "head_b" in opt_sb:
                    nc.vector.tensor_tensor(
                        out=sb[:], in0=sb[:],
                        in1=opt_sb["head_b"][:, j * c.V2:(j + 1) * c.V2],
                        op=ALU.add)
                nc.sync.dma_start(out=logits[j, t * P:(t + 1) * P, :],
                                  in_=sb[:])

    # ---------------- encoder ----------------
    qkv0 = qkv_fn("e_qkv_0", "e_qkv_b_0")
    qkv0.load()
    embed(codes_in)
    for hf in range(2):
        transpose_tokens(x_T, hf)
        qkv0(hf)
    for l in range(c.L):
        wts_out = load_out(f"e_out_{l}")
        attention(causal=False)
        ffn = ffn_fn(f"e_ff1_{l}", f"e_ff1_b_{l}",
                     f"e_ff2_{l}", f"e_ff2_b_{l}")
        out_proj_ar(wts_out, f"e_out_b_{l}")
        ffn.load()
        boundary(f"e_ln1_w_{l}", f"e_ln1_b_{l}", x_T, ffn)
        if l + 1 < c.L:
            nxt = qkv_fn(f"e_qkv_{l + 1}", f"e_qkv_b_{l + 1}")
            nxt.load()
            boundary(f"e_ln2_w_{l}", f"e_ln2_b_{l}", x_T, nxt)
        else:
            boundary(f"e_ln2_w_{l}", f"e_ln2_b_{l}", mem_T, None)

    # ---------------- decoder ----------------
    sqkv0 = qkv_fn("d_sqkv_0", "d_sqkv_b_0")
    sqkv0.load()
    embed(codes_tgt)
    for hf in range(2):
        transpose_tokens(x_T, hf)
        sqkv0(hf)
    for l in range(c.L):
        wts_sout = load_out(f"d_sout_{l}")
        cqk = load_qk(f"d_cqkv_qk_{l}")
        wv_c = load_wv(f"d_cqkv_{l}")
        attention(causal=True)
        out_proj_ar(wts_sout, f"d_sout_b_{l}")
        # cross-attention K/V depend only on encoder memory -- emitted here
        # so the TensorEngine stays busy while the AllReduce is in flight
        proj_full(cqk, f"d_cqkv_b_{l}", mem_T, c.DL, k_T)
        for hf in range(2):
            proj_v_half(mem_T, hf, wv_c)
        cq = lambda hf, _cqk=cqk, _l=l: proj_half(
            _cqk, f"d_cqkv_b_{_l}", x_T, 0, q_T, hf)
        wts_cout = load_out(f"d_cout_{l}")
        boundary(f"d_ln1_w_{l}", f"d_ln1_b_{l}", x_T, cq)
        attention(causal=False)
        ffn = ffn_fn(f"d_ff1_{l}", f"d_ff1_b_{l}",
                     f"d_ff2_{l}", f"d_ff2_b_{l}")
        out_proj_ar(wts_cout, f"d_cout_b_{l}")
        ffn.load()
        boundary(f"d_ln2_w_{l}", f"d_ln2_b_{l}", x_T, ffn)
        if l + 1 < c.L:
            nxt = qkv_fn(f"d_sqkv_{l + 1}", f"d_sqkv_b_{l + 1}")
            nxt.load()
            boundary(f"d_ln3_w_{l}", f"d_ln3_b_{l}", x_T, nxt)
        else:
            boundary(f"d_ln3_w_{l}", f"d_ln3_b_{l}", x_T, head_fn)

    es.close()


# --------------------------------------------------------------------------
# host side
# --------------------------------------------------------------------------

_PROG_CACHE = {}


def parse_cfg(inputs, TP=None, n_cores=None):
    B, Q, T = inputs["input_codes"].shape
    _, V, E = np.asarray(inputs["tok_emb"]).shape
    L, _, D = np.asarray(inputs["e_qkv_w"]).shape
    FF = np.asarray(inputs["e_ff1_w"]).shape[1]
    H = D // 64
    if TP is None:
        TP = int(os.environ.get("BASS_S2S_TP", "2"))
    if n_cores is None:
        n_cores = B * TP
    flags = set()
    for l in range(L):
        for ref, knm in [("e_qkv_b", "e_qkv_b"), ("d_sqkv_b", "d_sqkv_b"),
                         ("d_cqkv_b", "d_cqkv_b"), ("e_ff1_b", "e_ff1_b"),
                         ("d_ff1_b", "d_ff1_b"), ("e_out_b", "e_out_b"),
                         ("e_ff2_b", "e_ff2_b"), ("d_sout_b", "d_sout_b"),
                         ("d_cout_b", "d_cout_b"), ("d_ff2_b", "d_ff2_b")]:
            if np.any(np.asarray(inputs[ref])[l]):
                flags.add(f"{knm}_{l}")
        for ln in ["e_ln1", "e_ln2", "d_ln1", "d_ln2", "d_ln3"]:
            if not np.all(np.asarray(inputs[ln + "_w"])[l] == 1.0):
                flags.add(f"{ln}_w_{l}")
            if np.any(np.asarray(inputs[ln + "_b"])[l]):
                flags.add(f"{ln}_b_{l}")
    if np.any(np.asarray(inputs["head_b"])):
        flags.add("head_b")
    # v-bias unsupported in-kernel; fall back assertion
    for l in range(L):
        for nm in ["e_qkv_b", "d_sqkv_b", "d_cqkv_b"]:
            vb = np.asarray(inputs[nm])[l][2 * D:3 * D]
            assert not np.any(vb), "nonzero v bias not supported"
    return Cfg(B, Q, T, D, H, V, L, FF, TP, n_cores, flags)


def build_inmaps(inputs, c: Cfg):
    g = lambda nm: np.asarray(inputs[nm], np.float32)
    bf = lambda a: np.ascontiguousarray(a, dtype=np.float32).astype(BF16)

    tok = np.asarray(inputs["tok_emb"], np.float32)
    posf = np.ascontiguousarray(g("pos_emb")[0, :c.T, :])
    head_w = g("head_w")

    common = {f"tok_emb_{q}": np.ascontiguousarray(tok[q])
              for q in range(c.Q)}
    common["pos"] = posf

    def pretile(WT, n_tiles):
        """[D, n*128] -> [n, P, (D/P)*P] stationary tiles, each contiguous."""
        nd = WT.shape[0] // P
        r = WT.reshape(nd, P, WT.shape[1])
        return np.stack(
            [np.ascontiguousarray(
                r[:, :, j * P:(j + 1) * P].transpose(1, 0, 2).reshape(
                    P, nd * P)) for j in range(n_tiles)])

    per_tp = []
    for tp in range(c.TP):
        d = {}
        sl_d = slice(tp * c.DL, (tp + 1) * c.DL)
        sl_f = slice(tp * c.FFL, (tp + 1) * c.FFL)
        sl_v = slice(tp * c.V2, (tp + 1) * c.V2)
        head_t = np.concatenate([head_w[q].T[:, sl_v] for q in range(c.Q)],
                                axis=1)
        d["head_t"] = bf(head_t)
        if "head_b" in c.flags:
            hb = g("head_b")[:, sl_v].reshape(-1)
            d["head_b"] = np.broadcast_to(hb, (P, c.Q * c.V2)).copy()
        for pre, wq, wo in [
                ("e", "e_qkv_w", "e_out_w"),
                ("d_s", "d_sqkv_w", "d_sout_w"),
                ("d_c", "d_cqkv_w", "d_cout_w")]:
            qkv = g(wq)
            out_w = g(wo)
            for l in range(c.L):
                wqkv = np.concatenate(
                    [qkv[l][0:c.D][sl_d], qkv[l][c.D:2 * c.D][sl_d],
                     qkv[l][2 * c.D:3 * c.D][sl_d]], axis=0)
                nm = {"e": "e_qkv", "d_s": "d_sqkv", "d_c": "d_cqkv"}[pre]
                wqkv_t = bf(wqkv.T)
                d[f"{nm}_{l}"] = wqkv_t
                d[f"{nm}_qk_{l}"] = pretile(wqkv_t[:, :2 * c.DL], 2 * c.NQK)
                onm = {"e": "e_out", "d_s": "d_sout", "d_c": "d_cout"}[pre]
                d[f"{onm}_{l}"] = bf(out_w[l][:, sl_d].T)
        for l in range(c.L):
            d[f"e_ff1_t_{l}"] = pretile(bf(g("e_ff1_w")[l][sl_f].T), c.NFF)
            d[f"e_ff2_{l}"] = bf(g("e_ff2_w")[l][:, sl_f].T)
            d[f"d_ff1_t_{l}"] = pretile(bf(g("d_ff1_w")[l][sl_f].T), c.NFF)
            d[f"d_ff2_{l}"] = bf(g("d_ff2_w")[l][:, sl_f].T)
        # optional biases
        for l in range(c.L):
            for knm, ref in [("e_qkv_b", "e_qkv_b"), ("d_sqkv_b", "d_sqkv_b"),
                             ("d_cqkv_b", "d_cqkv_b")]:
                if f"{knm}_{l}" in c.flags:
                    b = g(ref)[l]
                    d[f"{knm}_{l}"] = np.concatenate(
                        [b[0:c.D][sl_d], b[c.D:2 * c.D][sl_d],
                         np.zeros(c.DL, np.float32)])
            for knm in ["e_ff1_b", "d_ff1_b"]:
                if f"{knm}_{l}" in c.flags:
                    d[f"{knm}_{l}"] = np.ascontiguousarray(g(knm + "")[l][sl_f])
            for knm in ["e_out_b", "e_ff2_b", "d_sout_b", "d_cout_b",
                        "d_ff2_b"]:
                if f"{knm}_{l}" in c.flags:
                    d[f"{knm}_{l}"] = np.broadcast_to(
                        g(knm)[l], (P, c.D)).copy()
            for ln in ["e_ln1", "e_ln2", "d_ln1", "d_ln2", "d_ln3"]:
                for sfx in ["w", "b"]:
                    if f"{ln}_{sfx}_{l}" in c.flags:
                        d[f"{ln}_{sfx}_{l}"] = np.broadcast_to(
                            g(f"{ln}_{sfx}")[l], (P, c.D)).copy()
        per_tp.append(d)

    codes_in = np.asarray(inputs["input_codes"], np.int32)
    codes_tgt = np.asarray(inputs["target_codes"], np.int32)
    in_maps = []
    for core in range(c.n_cores):
        b, tp = core // c.TP, core % c.TP
        m = dict(common)
        m.update(per_tp[tp])
        m["codes_in"] = np.ascontiguousarray(codes_in[b % c.B])
        m["codes_tgt"] = np.ascontiguousarray(codes_tgt[b % c.B])
        in_maps.append(m)
    return in_maps


def postprocess(results, c: Cfg):
    out = np.empty((c.B, c.T, c.Q, c.V), np.float32)
    for b in range(c.B):
        for tp in range(c.TP):
            r = results[b * c.TP + tp]["logits"]      # [Q, T, V2]
            out[b, :, :, tp * c.V2:(tp + 1) * c.V2] = r.transpose(1, 0, 2)
    return out


def run(inputs, trace=False):
    from concourse.bass_utils import run_bass_kernel_spmd
    c = parse_cfg(inputs)
    key = c.key()
    if key not in _PROG_CACHE:
        _PROG_CACHE[key] = build_program(c)
    nc = _PROG_CACHE[key]
    in_maps = build_inmaps(inputs, c)
    res = run_bass_kernel_spmd(nc, in_maps, list(range(c.n_cores)),
                               trace=trace)
    return postprocess(res.results, c), res


def kernel(**inputs):
    out, _ = run(inputs, trace=False)
    return out


# revision 31
# speedup vs baseline: 1.0278x; 1.0278x over previous
"""Trainium2 Bass kernel for an encoder-decoder (S2S) transformer.

Distribution: 8 NeuronCores = 4 data-parallel groups (batch B=4) x 2-way
Megatron tensor-parallel pairs.  Per TP pair, attention heads (qkv/out) and
ffn (ff1/ff2) are sharded; partials combine with a 2-core AllReduce after
the attention out-projection and after ff2.  The output head is split by
vocab half across the pair.

Matmuls run in bf16 on the TensorEngine (fp32 PSUM accumulation); the
residual stream is kept in bf16 (LN statistics in fp32).  Activations are
SBUF-resident for the whole pass; only weights stream from HBM.

Pipelining: every sublayer boundary is processed in two token halves.  The
AllReduce for half h is issued as soon as its producer matmuls finish, and
the LN + transpose + next-sublayer matmuls of half 0 execute while half 1's
AllReduce is still in flight, keeping the TensorEngine fed (the PE clock
ramps to full speed only after ~3us of continuous execution, so gaps are
doubly expensive).

Layout conventions (per core, P=128, T=512, HT=256):
  x_bf[t]    [P, D]      bf16  residual stream, tokens-on-partitions
  x_T_h[hf]  [P, ND*HT]  bf16  transposed half: chunk d cols [d*HT:(d+1)*HT]
  q_T/k_T    [P, NQK*T]  bf16  rows = sharded head dims, chunk m = heads 2m,2m+1
  v_aug[t]   [P, HL*128] bf16  per head 64 value cols + 64 ones cols
  attn_T     [P, NO*T]   bf16  context, transposed (rows = sharded dims)
  h_T_h[hf]  [P, NFF*HT] bf16  ffn hidden, transposed half
"""

import os
import sys

for _p in ("/opt/trn_rl_repo", "/root/.axon_site/_ro/trn_rl_repo"):
    if os.path.isdir(_p) and _p not in sys.path:
        sys.path.insert(0, _p)

import numpy as np
import ml_dtypes

import concourse.bass as bass
import concourse.bacc as bacc
import concourse.tile as tile
from concourse import mybir
from concourse.bass import IndirectOffsetOnAxis
from concourse.masks import make_identity

BF16 = ml_dtypes.bfloat16
F32 = mybir.dt.float32
BF = mybir.dt.bfloat16
I32 = mybir.dt.int32
AF = mybir.ActivationFunctionType
ALU = mybir.AluOpType
AX = mybir.AxisListType

P = 128


class Cfg:
    def __init__(self, B, Q, T, D, H, V, L, FF, TP, n_cores, flags=frozenset()):
        self.B, self.Q, self.T, self.D, self.H, self.V, self.L, self.FF = \
            B, Q, T, D, H, V, L, FF
        self.TP, self.n_cores = TP, n_cores
        self.E = D // Q
        self.HD = D // H
        assert self.HD == 64, "head packing assumes head_dim 64"
        assert self.E == P, "per-quantizer embedding dim must be 128"
        self.SCALE = 1.0 / float(np.sqrt(self.HD))
        self.DL = D // TP
        self.FFL = FF // TP
        self.HL = H // TP
        self.V2 = V // TP
        self.NT = T // P
        self.HT = self.T // 2
        self.ND = D // P
        self.NQK = self.DL // P
        self.NO = self.DL // P
        self.NFF = self.FFL // P
        assert self.HL % 2 == 0, "needs an even number of local heads"
        assert self.NT == 4, "token-half pipelining assumes 4 token blocks"
        self.flags = frozenset(flags)

    def key(self):
        return (self.B, self.Q, self.T, self.D, self.H, self.V, self.L,
                self.FF, self.TP, self.n_cores, tuple(sorted(self.flags)))


# --------------------------------------------------------------------------
# program builder
# --------------------------------------------------------------------------

def build_program(c: Cfg):
    nc = bacc.Bacc(None, target_bir_lowering=False, num_devices=c.n_cores)

    def din(name, shape, dt=BF):
        return nc.dram_tensor(name, shape, dt, kind="ExternalInput")

    codes_in = din("codes_in", [c.Q, c.T], I32)
    codes_tgt = din("codes_tgt", [c.Q, c.T], I32)
    tok_emb = [din(f"tok_emb_{q}", [c.V, c.E], F32) for q in range(c.Q)]
    pos = din("pos", [c.T, c.D], F32)

    w = {}
    for l in range(c.L):
        for nm, sh in [
                (f"e_qkv_{l}", [c.D, 3 * c.DL]), (f"e_out_{l}", [c.DL, c.D]),
                (f"e_ff2_{l}", [c.FFL, c.D]),
                (f"d_sqkv_{l}", [c.D, 3 * c.DL]), (f"d_sout_{l}", [c.DL, c.D]),
                (f"d_cqkv_{l}", [c.D, 3 * c.DL]), (f"d_cout_{l}", [c.DL, c.D]),
                (f"d_ff2_{l}", [c.FFL, c.D])]:
            w[nm] = din(nm, sh)
        # pre-tiled stationary weights: one contiguous [P, ND*P] block per
        # matmul tile so each weight DMA is a single cheap descriptor
        for nm in ("e_qkv", "d_sqkv", "d_cqkv"):
            w[f"{nm}_qk_{l}"] = din(f"{nm}_qk_{l}",
                                    [2 * c.NQK, P, c.ND * P])
        for nm in ("e_ff1", "d_ff1"):
            w[f"{nm}_t_{l}"] = din(f"{nm}_t_{l}", [c.NFF, P, c.ND * P])
    w["head_t"] = din("head_t", [c.E, c.Q * c.V2])

    opt = {}
    for nm in c.flags:
        if "_qkv_b_" in nm or "_sqkv_b_" in nm or "_cqkv_b_" in nm:
            opt[nm] = din(nm, [3 * c.DL], F32)
        elif "_ff1_b_" in nm:
            opt[nm] = din(nm, [c.FFL], F32)
        elif nm == "head_b":
            opt[nm] = din(nm, [P, c.Q * c.V2], F32)
        else:
            opt[nm] = din(nm, [P, c.D], F32)

    logits = nc.dram_tensor("logits", [c.Q, c.T, c.V2], F32,
                            kind="ExternalOutput")

    groups = ([[g * c.TP + i for i in range(c.TP)]
               for g in range(c.n_cores // c.TP)] if c.TP > 1 else None)

    with tile.TileContext(nc) as tc:
        _emit(nc, tc, c, codes_in, codes_tgt, tok_emb, pos, w, opt, logits,
              groups)
    nc.compile()
    return nc


def _emit(nc, tc, c, codes_in, codes_tgt, tok_emb, pos, w, opt, logits,
          groups):
    from contextlib import ExitStack
    es = ExitStack()
    pool = lambda name, bufs, space="SBUF": es.enter_context(
        tc.tile_pool(name=name, bufs=bufs, space=space))

    const = pool("const", 1)
    persist = pool("persist", 1)
    wpool = pool("wpool", 3)
    scratch = pool("scratch", 2)
    atp = pool("atp", 2)
    dram = pool("dram", 4, space="DRAM")
    ps_proj = pool("ps_proj", 2, space="PSUM")
    ps_s = pool("ps_s", 2, space="PSUM")
    ps_av = pool("ps_av", 2, space="PSUM")
    ps_tr = pool("ps_tr", 2, space="PSUM")

    HT = c.HT

    # constants
    ident_b = const.tile([P, P], BF, name="ident_b")
    make_identity(nc, ident_b[:])
    causT = const.tile([P, P], F32, name="causT")
    nc.gpsimd.memset(causT[:], 0.0)
    # transposed causal: fill -1e9 where tk > tq  (iota = tq - tk >= 0 keeps)
    nc.gpsimd.affine_select(out=causT[:], in_=causT[:],
                            compare_op=ALU.is_ge, fill=-1e9, base=0,
                            pattern=[[1, P]], channel_multiplier=-1)

    eps_t = const.tile([P, 1], F32, name="eps_t")
    nc.vector.memset(eps_t[:], 1e-5)

    opt_sb = {}
    for nm in opt:
        if "qkv_b_" in nm:
            t = const.tile([P, 3 * c.NQK], F32, name=f"sb_{nm}")
            nc.sync.dma_start(out=t[:],
                              in_=opt[nm].rearrange("(m p) -> p m", p=P))
        elif "_ff1_b_" in nm:
            t = const.tile([P, c.NFF], F32, name=f"sb_{nm}")
            nc.sync.dma_start(out=t[:],
                              in_=opt[nm].rearrange("(m p) -> p m", p=P))
        else:
            sh = [P, c.Q * c.V2] if nm == "head_b" else [P, c.D]
            t = const.tile(sh, F32, name=f"sb_{nm}")
            nc.sync.dma_start(out=t[:], in_=opt[nm][:])
        opt_sb[nm] = t

    # persistent activations
    x_bf = [persist.tile([P, c.D], BF, name=f"xb_{t}", tag=f"xb_{t}")
            for t in range(c.NT)]
    x_T = persist.tile([P, c.ND * c.T], BF, name="x_T", tag="x_T")
    mem_T = persist.tile([P, c.ND * c.T], BF, name="mem_T", tag="mem_T")
    q_T = persist.tile([P, c.NQK * c.T], BF, name="q_T", tag="q_T")
    k_T = persist.tile([P, c.NQK * c.T], BF, name="k_T", tag="k_T")
    WA = c.HL * 128   # per-t block: per head 64 ones cols + 64 value cols
    v_aug = [persist.tile([P, WA], BF, name=f"va_{t}", tag=f"va_{t}")
             for t in range(c.NT)]
    attn_T = persist.tile([P, c.NO * c.T], BF, name="attn_T", tag="attn_T")
    h_T = persist.tile([P, c.NFF * c.T], BF, name="h_T", tag="h_T")

    # ones columns of v_aug are set once; projections only write data cols.
    # ones come FIRST so the softmax denominator lands on partitions 0..63
    # (reciprocal_approx_fast needs a partition-aligned input).
    for t in range(c.NT):
        blk = v_aug[t][:].rearrange("p (h f) -> p h f", f=128)
        nc.vector.memset(blk[:, :, 0:64], 1.0)

    # psum-drain copies: route to the engine that is idle in that phase
    _cp = [0]

    def copy_out(dst, src, eng=None):
        if eng is None:
            eng = "s" if _cp[0] % 2 == 0 else "v"
            _cp[0] += 1
        if eng == "s":
            nc.scalar.copy(out=dst, in_=src)
        elif eng == "g":
            nc.gpsimd.tensor_copy(out=dst, in_=src)
        else:
            nc.vector.tensor_copy(out=dst, in_=src)

    # ---------------- helpers ----------------
    def transpose_tokens(dst, hf):
        """dst[:, d*T + t*P : +P] = x_bf[t][:, d*P:+P].T"""
        dv = dst[:].rearrange("p (nd x) -> p nd x", x=c.T)
        for t in (2 * hf, 2 * hf + 1):
            for dh in range(c.ND // 4):
                pt = ps_tr.tile([P, 4 * P], BF, tag="ps_tr", name="pt")
                for j in range(4):
                    d = 4 * dh + j
                    nc.tensor.transpose(pt[:, j * P:(j + 1) * P],
                                        x_bf[t][:, d * P:(d + 1) * P],
                                        ident_b[:])
                copy_out(dv[:, 4 * dh:4 * dh + 4, t * P:(t + 1) * P],
                         pt[:].rearrange("p (d x) -> p d x", x=P))

    def embed(codes):
        for t in range(c.NT):
            ptile = scratch.tile([P, c.D], F32, tag="pos", name="ptile",
                                 bufs=1)
            nc.sync.dma_start(out=ptile[:], in_=pos[t * P:(t + 1) * P, :])
            e_t = scratch.tile([P, c.D], F32, tag="emb", name="e_t", bufs=1)
            for q in range(c.Q):
                idx = scratch.tile([P, 1], I32, tag="idx", name="idx", bufs=8)
                nc.sync.dma_start(out=idx[:], in_=codes[q, t * P:(t + 1) * P])
                nc.gpsimd.indirect_dma_start(
                    out=e_t[:, q * c.E:(q + 1) * c.E],
                    out_offset=None,
                    in_=tok_emb[q][:],
                    in_offset=IndirectOffsetOnAxis(ap=idx[:, :1], axis=0))
            nc.vector.tensor_tensor(out=e_t[:], in0=e_t[:], in1=ptile[:],
                                    op=ALU.add)
            copy_out(x_bf[t][:], e_t[:], eng="s")

    def ln_chunk(t, sub, gname, bname):
        """x_bf[t] = LN(x_bf[t] + sub) * g + b (post-norm), bf16 residual."""
        tmp = scratch.tile([P, c.D], F32, tag="lntmp", name="tmp", bufs=2)
        nc.vector.tensor_tensor(out=tmp[:], in0=x_bf[t][:], in1=sub,
                                op=ALU.add)
        s1 = scratch.tile([P, 1], F32, tag="lnstat", name="s1", bufs=8)
        s2 = scratch.tile([P, 1], F32, tag="lnstat", name="s2", bufs=8)
        sq = scratch.tile([P, c.D], BF, tag="lnsq", name="sq", bufs=2)
        nc.vector.reduce_sum(out=s1[:], in_=tmp[:], axis=AX.X)
        nc.scalar.activation(sq[:], tmp[:], AF.Square, accum_out=s2[:])
        mean = scratch.tile([P, 1], F32, tag="lnstat", name="mean", bufs=8)
        var = scratch.tile([P, 1], F32, tag="lnstat", name="var", bufs=8)
        m2 = scratch.tile([P, 1], F32, tag="lnstat", name="m2", bufs=8)
        nc.vector.tensor_scalar_mul(mean[:], s1[:], 1.0 / c.D)
        nc.vector.tensor_scalar_mul(var[:], s2[:], 1.0 / c.D)
        nc.vector.tensor_tensor(out=m2[:], in0=mean[:], in1=mean[:],
                                op=ALU.mult)
        nc.vector.tensor_tensor(out=var[:], in0=var[:], in1=m2[:],
                                op=ALU.subtract)
        rstd = scratch.tile([P, 1], F32, tag="lnstat", name="rstd", bufs=8)
        nc.scalar.activation(rstd[:], var[:], AF.Sqrt, bias=eps_t[:])
        nc.vector.reciprocal(rstd[:], rstd[:])
        if gname in opt_sb or bname in opt_sb:
            nc.vector.tensor_scalar(out=tmp[:], in0=tmp[:], scalar1=mean[:],
                                    scalar2=rstd[:], op0=ALU.subtract,
                                    op1=ALU.mult)
            if gname in opt_sb:
                nc.vector.tensor_tensor(out=tmp[:], in0=tmp[:],
                                        in1=opt_sb[gname][:], op=ALU.mult)
            if bname in opt_sb:
                nc.vector.tensor_tensor(out=tmp[:], in0=tmp[:],
                                        in1=opt_sb[bname][:], op=ALU.add)
            nc.vector.tensor_copy(out=x_bf[t][:], in_=tmp[:])
        else:
            nc.vector.tensor_scalar(out=x_bf[t][:], in0=tmp[:],
                                    scalar1=mean[:], scalar2=rstd[:],
                                    op0=ALU.subtract, op1=ALU.mult)

    red_tiles = [None, None]

    def ar_start(ph, hf):
        """Kick off the TP AllReduce for token half hf of a sublayer."""
        if groups is None:
            red_tiles[hf] = ph
            return
        arin = dram.tile([P, 2 * c.D], BF, tag="arin", name="arin", bufs=4)
        arout = dram.tile([P, 2 * c.D], BF, tag="arout", name="arout", bufs=4)
        nc.sync.dma_start(out=arin[:], in_=ph[:])
        nc.gpsimd.collective_compute(
            "AllReduce", ALU.add, replica_groups=groups,
            ins=[arin[:].opt()], outs=[arout[:].opt()])
        red = scratch.tile([P, 2 * c.D], BF, tag="ar_red", name="red", bufs=2)
        nc.sync.dma_start(out=red[:], in_=arout[:])
        red_tiles[hf] = red

    def ln_finish(hf, gname, bname):
        red = red_tiles[hf]
        for i, t in enumerate((2 * hf, 2 * hf + 1)):
            ln_chunk(t, red[:, i * c.D:(i + 1) * c.D], gname, bname)

    def boundary(gname, bname, dst_pair, next_fn):
        """AR results -> LN -> transpose -> next sublayer, per token half."""
        for hf in range(2):
            ln_finish(hf, gname, bname)
            transpose_tokens(dst_pair, hf)
            if next_fn is not None:
                next_fn(hf)

    def load_qk(tname, n0=0, n1=None):
        """Load pre-tiled stationary weight blocks (one DMA per tile)."""
        if n1 is None:
            n1 = 2 * c.NQK
        tiles = {}
        for j in range(n0, n1):
            wt = wpool.tile([P, c.ND * P], BF, tag=f"wqk_{j}", name="wt",
                            bufs=1)
            nc.sync.dma_start(out=wt[:], in_=w[tname][j])
            tiles[j] = wt
        return tiles

    def proj_half(qk_tiles, bname, src, row_base, dst, hf):
        """dst[:, m*T + hf*HT : +HT] = rows [row_base + m*P ..] of W.T @ src."""
        for m in range(c.NQK):
            mg = row_base // P + m
            wt = qk_tiles[mg]
            ps = ps_proj.tile([P, 512], F32, tag="ps_proj", name="ps")
            for k in range(c.ND):
                nc.tensor.matmul(ps[:, :HT], wt[:, k * P:(k + 1) * P],
                                 src[:, k * c.T + hf * HT:
                                     k * c.T + (hf + 1) * HT],
                                 start=(k == 0), stop=(k == c.ND - 1))
            col = m * c.T + hf * HT
            if bname in opt_sb:
                nc.vector.tensor_scalar(
                    out=dst[:, col:col + HT], in0=ps[:, :HT],
                    scalar1=opt_sb[bname][:, mg:mg + 1], scalar2=None,
                    op0=ALU.add)
            else:
                copy_out(dst[:, col:col + HT], ps[:, :HT], eng="v")

    def proj_v_half(src, hf, wv_tiles):
        for t in (2 * hf, 2 * hf + 1):
            ps = ps_proj.tile([P, 512], F32, tag="ps_proj", name="ps")
            for k in range(c.ND):
                nc.tensor.matmul(
                    ps[:, :c.DL],
                    src[:, k * c.T + t * P: k * c.T + t * P + P],
                    wv_tiles[k][:], start=(k == 0), stop=(k == c.ND - 1))
            blk = v_aug[t][:].rearrange("p (h f) -> p h f", f=128)
            copy_out(blk[:, :, 64:128],
                     ps[:, :c.DL].rearrange("p (h f) -> p h f", f=64))

    def load_wv(wname):
        wv_tiles = []
        for k in range(c.ND):
            wt = wpool.tile([P, c.DL], BF, tag=f"wv_{k}", name="wt", bufs=1)
            nc.sync.dma_start(
                out=wt[:],
                in_=w[wname][k * P:(k + 1) * P, 2 * c.DL:3 * c.DL])
            wv_tiles.append(wt)
        return wv_tiles

    def qkv_fn(wname, bname):
        state = {}
        base, l = wname.rsplit("_", 1)

        def load():
            if "qk" not in state:
                state["qk"] = load_qk(f"{base}_qk_{l}")
                state["wv"] = load_wv(wname)

        def go(hf):
            if hf == 0:
                load()
            proj_half(state["qk"], bname, x_T, 0, q_T, hf)
            proj_half(state["qk"], bname, x_T, c.DL, k_T, hf)
            proj_v_half(x_T, hf, state["wv"])
        go.load = load
        return go

    def attention(causal):
        """q_T,k_T,v_aug -> attn_T, pipelined one head deep so the PE does
        head h+1's scores while the scalar engine exponentiates head h."""
        def emit_scores(hh):
            m, po = hh // 2, 64 * (hh % 2)
            ats = []
            for tk in range(c.NT):
                tq0 = tk * P if causal else 0
                pss = ps_s.tile([P, c.T], F32, tag="ps_s", name="pss")
                at = atp.tile([P, c.T], BF, tag=f"AT{tk}", name="at", bufs=2)
                nc.tensor.matmul(
                    pss[:, tq0:c.T],
                    k_T[po:po + 64, m * c.T + tk * P: m * c.T + tk * P + P],
                    q_T[po:po + 64, m * c.T + tq0: m * c.T + c.T],
                    start=True, stop=True)
                if causal:
                    nc.vector.tensor_tensor(
                        out=pss[:, tk * P:(tk + 1) * P],
                        in0=pss[:, tk * P:(tk + 1) * P],
                        in1=causT[:], op=ALU.add)
                nc.scalar.activation(at[:, tq0:c.T], pss[:, tq0:c.T],
                                     AF.Exp, scale=c.SCALE)
                ats.append((at, tq0))
            return ats

        def emit_av(hh, ats):
            m, po = hh // 2, 64 * (hh % 2)
            ps_o = ps_av.tile([P, c.T], F32, tag="ps_av", name="ps_o")
            for tk in range(c.NT):
                cols0 = ats[tk][1]
                nc.tensor.matmul(
                    ps_o[:, cols0:c.T],
                    v_aug[tk][:, hh * 128:(hh + 1) * 128],
                    ats[tk][0][:, cols0:c.T],
                    start=(tk == 0), stop=(tk == c.NT - 1))
            rden = scratch.tile([64, c.T], F32, tag="rden", name="rden",
                                bufs=2)
            nc.vector.reciprocal_approx_fast(rden[:], ps_o[0:64, :])
            nc.vector.tensor_tensor(
                out=attn_T[po:po + 64, m * c.T:(m + 1) * c.T],
                in0=ps_o[64:128, :], in1=rden[:], op=ALU.mult)

        prev = None
        for hh in range(c.HL):
            ats = emit_scores(hh)
            if prev is not None:
                emit_av(*prev)
            prev = (hh, ats)
        emit_av(*prev)

    NB = max(c.D // 512, 1)
    NW = min(512, c.D)

    def natural_half_ar(src_T_of, nk, wts, bname, hf):
        """parts[tokens of half hf, :D] = src^T @ W; then start the AR."""
        ph = scratch.tile([P, 2 * c.D], BF, tag="oproj", name="ph", bufs=2)
        for i, t in enumerate((2 * hf, 2 * hf + 1)):
            for n in range(NB):
                ps = ps_proj.tile([P, 512], F32, tag="ps_proj", name="ps")
                for k in range(nk):
                    nc.tensor.matmul(
                        ps[:], src_T_of(k, t),
                        wts[k][:, n * NW:(n + 1) * NW],
                        start=(k == 0), stop=(k == nk - 1))
                copy_out(ph[:, i * c.D + n * NW: i * c.D + (n + 1) * NW],
                         ps[:], eng="v")
        if bname in opt_sb:
            for i in range(2):
                nc.vector.tensor_tensor(
                    out=ph[:, i * c.D:(i + 1) * c.D],
                    in0=ph[:, i * c.D:(i + 1) * c.D],
                    in1=opt_sb[bname][:], op=ALU.add)
        ar_start(ph, hf)

    def load_out(wname):
        wts = []
        for k in range(c.NO):
            wt = wpool.tile([P, c.D], BF, tag=f"wo_{k}", name="wt", bufs=1)
            nc.sync.dma_start(out=wt[:], in_=w[wname][k * P:(k + 1) * P, :])
            wts.append(wt)
        return wts

    def out_proj_ar(wts, bname):
        src = lambda k, t: attn_T[:, k * c.T + t * P: k * c.T + t * P + P]
        for hf in range(2):
            natural_half_ar(src, c.NO, wts, bname, hf)

    NW1 = 6   # ff1 weight-tile rotation width

    def ffn_fn(w1name, b1name, w2name, b2name):
        state = {}
        base, l = w1name.rsplit("_", 1)
        w1t = f"{base}_t_{l}"

        def load():
            if "pre" in state:
                return
            # prefetch the first ff1 tiles, then the resident ff2 tiles, so
            # both transfer during the attention-out AllReduce
            pre = []
            for m in range(NW1):
                wt = wpool.tile([P, c.ND * P], BF, tag=f"wf1_{m % NW1}",
                                name="wt", bufs=2)
                nc.sync.dma_start(out=wt[:], in_=w[w1t][m])
                pre.append(wt)
            state["pre"] = pre
            wts = []
            for k in range(c.NFF):
                wt = wpool.tile([P, c.D], BF, tag=f"wff2_{k}", name="wt",
                                bufs=1)
                nc.sync.dma_start(out=wt[:],
                                  in_=w[w2name][k * P:(k + 1) * P, :])
                wts.append(wt)
            state["w2"] = wts

        def go(hf):
            if hf == 0:
                load()
            for m in range(c.NFF):
                if hf == 0 and m < NW1:
                    wt = state["pre"][m]
                else:
                    wt = wpool.tile([P, c.ND * P], BF, tag=f"wf1_{m % NW1}",
                                    name="wt", bufs=2)
                    nc.sync.dma_start(out=wt[:], in_=w[w1t][m])
                ps = ps_proj.tile([P, 512], F32, tag="ps_proj", name="ps")
                for k in range(c.ND):
                    nc.tensor.matmul(ps[:, :HT], wt[:, k * P:(k + 1) * P],
                                     x_T[:, k * c.T + hf * HT:
                                         k * c.T + (hf + 1) * HT],
                                     start=(k == 0), stop=(k == c.ND - 1))
                col = m * c.T + hf * HT
                if b1name in opt_sb:
                    nc.scalar.activation(h_T[:, col:col + HT],
                                         ps[:, :HT], AF.Relu,
                                         bias=opt_sb[b1name][:, m:m + 1])
                else:
                    nc.scalar.activation(h_T[:, col:col + HT],
                                         ps[:, :HT], AF.Relu)
            src = lambda k, t: h_T[:, k * c.T + t * P: k * c.T + t * P + P]
            natural_half_ar(src, c.NFF, state["w2"], b2name, hf)
        go.load = load
        return go

    def head_fn(hf):
        for j in range(c.Q):
            hw = wpool.tile([P, c.V2], BF, tag="whead", name="hw", bufs=2)
            nc.sync.dma_start(out=hw[:],
                              in_=w["head_t"][:, j * c.V2:(j + 1) * c.V2])
            for t in (2 * hf, 2 * hf + 1):
                ps = ps_proj.tile([P, 512], F32, tag="ps_proj", name="ps")
                nc.tensor.matmul(
                    ps[:, :c.V2],
                    x_T[:, j * c.T + t * P: j * c.T + t * P + P],
                    hw[:], start=True, stop=True)
                sb = scratch.tile([P, c.V2], F32, tag="lgt", name="sb",
                                  bufs=2)
                copy_out(sb[:], ps[:, :c.V2])
                if "head_b" in opt_sb:
                    nc.vector.tensor_tensor(
                        out=sb[:], in0=sb[:],
                        in1=opt_sb["head_b"][:, j * c.V2:(j + 1) * c.V2],
                        op=ALU.add)
                nc.sync.dma_start(out=logits[j, t * P:(t + 1) * P, :],
                                  in_=sb[:])

    # ---------------- encoder ----------------
    qkv0 = qkv_fn("e_qkv_0", "e_qkv_b_0")
    qkv0.load()
    embed(codes_in)
    for hf in range(2):
        transpose_tokens(x_T, hf)
        qkv0(hf)
    for l in range(c.L):
        wts_out = load_out(f"e_out_{l}")
        attention(causal=False)
        ffn = ffn_fn(f"e_ff1_{l}", f"e_ff1_b_{l}",
                     f"e_ff2_{l}", f"e_ff2_b_{l}")
        out_proj_ar(wts_out, f"e_out_b_{l}")
        ffn.load()
        boundary(f"e_ln1_w_{l}", f"e_ln1_b_{l}", x_T, ffn)
        if l + 1 < c.L:
            nxt = qkv_fn(f"e_qkv_{l + 1}", f"e_qkv_b_{l + 1}")
            nxt.load()
            boundary(f"e_ln2_w_{l}", f"e_ln2_b_{l}", x_T, nxt)
        else:
            boundary(f"e_ln2_w_{l}", f"e_ln2_b_{l}", mem_T, None)

    # ---------------- decoder ----------------
    sqkv0 = qkv_fn("d_sqkv_0", "d_sqkv_b_0")
    sqkv0.load()
    embed(codes_tgt)
    for hf in range(2):
        transpose_tokens(x_T, hf)
        sqkv0(hf)
    for l in range(c.L):
        wts_sout = load_out(f"d_sout_{l}")
        cqk = load_qk(f"d_cqkv_qk_{l}")
        wv_c = load_wv(f"d_cqkv_{l}")
        attention(causal=True)
        out_proj_ar(wts_sout, f"d_sout_b_{l}")
        # cross-attention K/V depend only on encoder memory -- emitted here
        # so the TensorEngine stays busy while the AllReduce is in flight
        for hf in range(2):
            proj_half(cqk, f"d_cqkv_b_{l}", mem_T, c.DL, k_T, hf)
            proj_v_half(mem_T, hf, wv_c)
        cq = lambda hf, _cqk=cqk, _l=l: proj_half(
            _cqk, f"d_cqkv_b_{_l}", x_T, 0, q_T, hf)
        wts_cout = load_out(f"d_cout_{l}")
        boundary(f"d_ln1_w_{l}", f"d_ln1_b_{l}", x_T, cq)
        attention(causal=False)
        ffn = ffn_fn(f"d_ff1_{l}", f"d_ff1_b_{l}",
                     f"d_ff2_{l}", f"d_ff2_b_{l}")
        out_proj_ar(wts_cout, f"d_cout_b_{l}")
        ffn.load()
        boundary(f"d_ln2_w_{l}", f"d_ln2_b_{l}", x_T, ffn)
        if l + 1 < c.L:
            nxt = qkv_fn(f"d_sqkv_{l + 1}", f"d_sqkv_b_{l + 1}")
            nxt.load()
            boundary(f"d_ln3_w_{l}", f"d_ln3_b_{l}", x_T, nxt)
        else:
            boundary(f"d_ln3_w_{l}", f"d_ln3_b_{l}", x_T, head_fn)

    es.close()


# --------------------------------------------------------------------------
# host side
# --------------------------------------------------------------------------

_PROG_CACHE = {}


def parse_cfg(inputs, TP=None, n_cores=None):
    B, Q, T = inputs["input_codes"].shape
    _, V, E = np.asarray(inputs["tok_emb"]).shape
    L, _, D = np.asarray(inputs["e_qkv_w"]).shape
    FF = np.asarray(inputs["e_ff1_w"]).shape[1]
    H = D // 64
    if TP is None:
        TP = int(os.environ.get("BASS_S2S_TP", "2"))
    if n_cores is None:
        n_cores = B * TP
    flags = set()
    for l in range(L):
        for ref, knm in [("e_qkv_b", "e_qkv_b"), ("d_sqkv_b", "d_sqkv_b"),
                         ("d_cqkv_b", "d_cqkv_b"), ("e_ff1_b", "e_ff1_b"),
                         ("d_ff1_b", "d_ff1_b"), ("e_out_b", "e_out_b"),
                         ("e_ff2_b", "e_ff2_b"), ("d_sout_b", "d_sout_b"),
                         ("d_cout_b", "d_cout_b"), ("d_ff2_b", "d_ff2_b")]:
            if np.any(np.asarray(inputs[ref])[l]):
                flags.add(f"{knm}_{l}")
        for ln in ["e_ln1", "e_ln2", "d_ln1", "d_ln2", "d_ln3"]:
            if not np.all(np.asarray(inputs[ln + "_w"])[l] == 1.0):
                flags.add(f"{ln}_w_{l}")
            if np.any(np.asarray(inputs[ln + "_b"])[l]):
                flags.add(f"{ln}_b_{l}")
    if np.any(np.asarray(inputs["head_b"])):
        flags.add("head_b")
    # v-bias unsupported in-kernel; fall back assertion
    for l in range(L):
        for nm in ["e_qkv_b", "d_sqkv_b", "d_cqkv_b"]:
            vb = np.asarray(inputs[nm])[l][2 * D:3 * D]
            assert not np.any(vb), "nonzero v bias not supported"
    return Cfg(B, Q, T, D, H, V, L, FF, TP, n_cores, flags)


def build_inmaps(inputs, c: Cfg):
    g = lambda nm: np.asarray(inputs[nm], np.float32)
    bf = lambda a: np.ascontiguousarray(a, dtype=np.float32).astype(BF16)

    tok = np.asarray(inputs["tok_emb"], np.float32)
    posf = np.ascontiguousarray(g("pos_emb")[0, :c.T, :])
    head_w = g("head_w")

    common = {f"tok_emb_{q}": np.ascontiguousarray(tok[q])
              for q in range(c.Q)}
    common["pos"] = posf

    def pretile(WT, n_tiles):
        """[D, n*128] -> [n, P, (D/P)*P] stationary tiles, each contiguous."""
        nd = WT.shape[0] // P
        r = WT.reshape(nd, P, WT.shape[1])
        return np.stack(
            [np.ascontiguousarray(
                r[:, :, j * P:(j + 1) * P].transpose(1, 0, 2).reshape(
                    P, nd * P)) for j in range(n_tiles)])

    per_tp = []
    for tp in range(c.TP):
        d = {}
        sl_d = slice(tp * c.DL, (tp + 1) * c.DL)
        sl_f = slice(tp * c.FFL, (tp + 1) * c.FFL)
        sl_v = slice(tp * c.V2, (tp + 1) * c.V2)
        head_t = np.concatenate([head_w[q].T[:, sl_v] for q in range(c.Q)],
                                axis=1)
        d["head_t"] = bf(head_t)
        if "head_b" in c.flags:
            hb = g("head_b")[:, sl_v].reshape(-1)
            d["head_b"] = np.broadcast_to(hb, (P, c.Q * c.V2)).copy()
        for pre, wq, wo in [
                ("e", "e_qkv_w", "e_out_w"),
                ("d_s", "d_sqkv_w", "d_sout_w"),
                ("d_c", "d_cqkv_w", "d_cout_w")]:
            qkv = g(wq)
            out_w = g(wo)
            for l in range(c.L):
                wqkv = np.concatenate(
                    [qkv[l][0:c.D][sl_d], qkv[l][c.D:2 * c.D][sl_d],
                     qkv[l][2 * c.D:3 * c.D][sl_d]], axis=0)
                nm = {"e": "e_qkv", "d_s": "d_sqkv", "d_c": "d_cqkv"}[pre]
                wqkv_t = bf(wqkv.T)
                d[f"{nm}_{l}"] = wqkv_t
                d[f"{nm}_qk_{l}"] = pretile(wqkv_t[:, :2 * c.DL], 2 * c.NQK)
                onm = {"e": "e_out", "d_s": "d_sout", "d_c": "d_cout"}[pre]
                d[f"{onm}_{l}"] = bf(out_w[l][:, sl_d].T)
        for l in range(c.L):
            d[f"e_ff1_t_{l}"] = pretile(bf(g("e_ff1_w")[l][sl_f].T), c.NFF)
            d[f"e_ff2_{l}"] = bf(g("e_ff2_w")[l][:, sl_f].T)
            d[f"d_ff1_t_{l}"] = pretile(bf(g("d_ff1_w")[l][sl_f].T), c.NFF)
            d[f"d_ff2_{l}"] = bf(g("d_ff2_w")[l][:, sl_f].T)
        # optional biases
        for l in range(c.L):
            for knm, ref in [("e_qkv_b", "e_qkv_b"), ("d_sqkv_b", "d_sqkv_b"),
                             ("d_cqkv_b", "d_cqkv_b")]:
                if f"{knm}_{l}" in c.flags:
                    b = g(ref)[l]
                    d[f"{knm}_{l}"] = np.concatenate(
                        [b[0:c.D][sl_d], b[c.D:2 * c.D][sl_d],
                         np.zeros(c.DL, np.float32)])
            for knm in ["e_ff1_b", "d_ff1_b"]:
                if f"{knm}_{l}" in c.flags:
                    d[f"{knm}_{l}"] = np.ascontiguousarray(g(knm + "")[l][sl_f])
            for knm in ["e_out_b", "e_ff2_b", "d_sout_b", "d_cout_b",
                        "d_ff2_b"]:
                if f"{knm}_{l}" in c.flags:
                    d[f"{knm}_{l}"] = np.broadcast_to(
                        g(knm)[l], (P, c.D)).copy()
            for ln in ["e_ln1", "e_ln2", "d_ln1", "d_ln2", "d_ln3"]:
                for sfx in ["w", "b"]:
                    if f"{ln}_{sfx}_{l}" in c.flags:
                        d[f"{ln}_{sfx}_{l}"] = np.broadcast_to(
                            g(f"{ln}_{sfx}")[l], (P, c.D)).copy()
        per_tp.append(d)

    codes_in = np.asarray(inputs["input_codes"], np.int32)
    codes_tgt = np.asarray(inputs["target_codes"], np.int32)
    in_maps = []
    for core in range(c.n_cores):
        b, tp = core // c.TP, core % c.TP
        m = dict(common)
        m.update(per_tp[tp])
        m["codes_in"] = np.ascontiguousarray(codes_in[b % c.B])
        m["codes_tgt"] = np.ascontiguousarray(codes_tgt[b % c.B])
        in_maps.append(m)
    return in_maps


def postprocess(results, c: Cfg):
    out = np.empty((c.B, c.T, c.Q, c.V), np.float32)
    for b in range(c.B):
        for tp in range(c.TP):
            r = results[b * c.TP + tp]["logits"]      # [Q, T, V2]
            out[b, :, :, tp * c.V2:(tp + 1) * c.V2] = r.transpose(1, 0, 2)
    return out


def run(inputs, trace=False):
    from concourse.bass_utils import run_bass_kernel_spmd
    c = parse_cfg(inputs)
    key = c.key()
    if key not in _PROG_CACHE:
        _PROG_CACHE[key] = build_program(c)
    nc = _PROG_CACHE[key]
    in_maps = build_inmaps(inputs, c)
    res = run_bass_kernel_spmd(nc, in_maps, list(range(c.n_cores)),
                               trace=trace)
    return postprocess(res.results, c), res


def kernel(**inputs):
    out, _ = run(inputs, trace=False)
    return out
